# revision 1
# baseline (speedup 1.0000x reference)
"""Trainium2 Bass kernel for ConditionalAttentionFusion-v2.

Math (per batch b, channel c, pixel y,x):
    CD   = concat(rgb_var, d_var)                       # [2,H,W], shared
    AB   = Wp[c,0]*rgb + Wp[c,1]*d
    CDc  = conv3x3(CD, W_unc[c])                        # 2-in 1-out per channel
    G    = Wt[c,0]*AB + Wt[c,1]*CDc
    out  = rgb*G + d*(1-G) = d + (rgb-d)*G

Strategy: pure data parallel over 8 cores (core = (batch, H-half), slab of 256
rows).  On each core the 3x3 conv (y-taps) + per-channel 1x1 terms are computed
on the TensorEngine as banded/diagonal-matrix matmuls accumulating into PSUM:

    G[r, x] = sum_{i,kx} Band[c,i,kx].T @ V_i[:, x+kx]    (6 band matmuls)
            + diag(a0[c]).T @ rgb + diag(a1[c]).T @ d     (2 diag matmuls)

where Band[p=r+ky, m=r] = Wt[c,1]*W_unc[c,i,ky,kx] folds the three ky taps of
the conv into one matmul (output rows 0..125 valid per 128-row V tile).  The
x-shifts (kx) are free-dim offsets into an x-padded V tile; the y-halo is
handled host-side by padding the var slab.  VectorE then does the 3-op tail:
diff = rgb-d; P = diff*G(PSUM); out = P + d.

A slab of 256 rows = two 126-row band tiles + a 4-row remainder.  The
remainder stacks all 19 channels into one matmul group (output partition
m = 4c+r), so it costs only 6 band + 2 diag matmuls total.

All band/diag matrices are precomputed host-side in numpy from the runtime
weight tensors and passed as extra kernel inputs.

Precision: the band (conv) matmuls use float32r (single-pass, bf16-array
speed) since the conv term is small; the diag matmuls on rgb/d use exact
float32 (2-pass) since those terms dominate the output magnitude.  Measured
absmax error ~1.7e-3 on an output scale of ~26 (6.5e-5 scale-relative).
"""
import sys

if "/opt/trn_rl_repo" not in sys.path:
    sys.path.insert(0, "/opt/trn_rl_repo")

import numpy as np

import concourse.bacc as bacc
import concourse.mybir as mybir
import concourse.tile as tile
from concourse.bass_utils import run_bass_kernel_spmd

F32 = mybir.dt.float32
F32R = mybir.dt.float32r
B, C, H, W = 4, 19, 512, 1024
R = 256              # slab rows per core
NCORES = 8
MAIN_Y0 = (0, 126)   # 126-row band tiles
REM_Y0 = 252         # 4-row remainder, stacked over channels


# ----------------------------------------------------------------- host math
def _build_mats(W_prob, W_unc, W_total):
    a0 = W_total[:, 0] * W_prob[:, 0]
    a1 = W_total[:, 0] * W_prob[:, 1]
    Wp = W_total[:, 1][:, None, None, None] * W_unc          # [C,2,3,3]

    bands = np.zeros((C, 128, 6, 128), np.float32)           # [c,p,s,m]
    r = np.arange(126)
    for i in range(2):
        for kx in range(3):
            s = i * 3 + kx
            for ky in range(3):
                bands[:, r + ky, s, r] = Wp[:, i, ky, kx][:, None]

    diags = np.zeros((C, 128, 2, 128), np.float32)           # [c,p,j,m]
    m = np.arange(128)
    diags[:, m, 0, m] = a0[:, None]
    diags[:, m, 1, m] = a1[:, None]

    remb = np.zeros((6, 6, 128), np.float32)                 # [p,s,m], m=4c+r
    rr = np.arange(4)
    for i in range(2):
        for kx in range(3):
            s = i * 3 + kx
            for ky in range(3):
                for c in range(C):
                    remb[rr + ky, s, 4 * c + rr] = Wp[c, i, ky, kx]

    remd = np.zeros((76, 2, 76), np.float32)                 # [p,j,m], p=m=4c+r
    p = np.arange(76)
    remd[p, 0, p] = np.repeat(a0, 4)
    remd[p, 1, p] = np.repeat(a1, 4)

    return (bands.reshape(C, 128, 768), diags.reshape(C, 128, 256),
            remb.reshape(6, 768), remd.reshape(76, 152))


# ------------------------------------------------------------- bass program
_CACHE = {}


def _build_program():
    nc = bacc.Bacc("TRN2", debug=False, num_devices=NCORES)
    f = F32R
    rgb_s = nc.dram_tensor("rgb_s", [C, R, W], F32, kind="ExternalInput").ap()
    d_s = nc.dram_tensor("d_s", [C, R, W], F32, kind="ExternalInput").ap()
    var_s = nc.dram_tensor("var_s", [2, R + 2, W + 2], f, kind="ExternalInput").ap()
    bands = nc.dram_tensor("bands", [C, 128, 768], f, kind="ExternalInput").ap()
    diags = nc.dram_tensor("diags", [C, 128, 256], F32, kind="ExternalInput").ap()
    remb = nc.dram_tensor("remb", [6, 768], f, kind="ExternalInput").ap()
    remd = nc.dram_tensor("remd", [76, 152], F32, kind="ExternalInput").ap()
    out_s = nc.dram_tensor("out_s", [C, R, W], F32, kind="ExternalOutput").ap()

    with tile.TileContext(nc) as tc:
        with (
            tc.tile_pool(name="wpool", bufs=1) as wpool,
            tc.tile_pool(name="vpool", bufs=4) as vpool,
            tc.tile_pool(name="io", bufs=3) as io,
            tc.tile_pool(name="tmp", bufs=2) as tmp,
            tc.tile_pool(name="psum", bufs=4, space="PSUM") as psum,
        ):
            band_sb, diag_sb = [], []
            for c in range(C):
                bt = wpool.tile([128, 768], f, tag=f"band{c}", name=f"band{c}")
                nc.sync.dma_start(out=bt[:], in_=bands[c])
                dt_ = wpool.tile([128, 256], F32, tag=f"diag{c}", name=f"diag{c}")
                nc.sync.dma_start(out=dt_[:], in_=diags[c])
                band_sb.append(bt)
                diag_sb.append(dt_)
            remb_sb = wpool.tile([6, 768], f, tag="remb", name="remb_sb")
            nc.sync.dma_start(out=remb_sb[:], in_=remb[:])
            remd_sb = wpool.tile([76, 152], F32, tag="remd", name="remd_sb")
            nc.sync.dma_start(out=remd_sb[:], in_=remd[:])

            # ---------------- main 126-row band tiles
            for y0 in MAIN_Y0:
                vt = []
                for i in range(2):
                    v = vpool.tile([128, W + 2], f, tag="v", name=f"v{i}_{y0}")
                    nc.sync.dma_start(out=v[:], in_=var_s[i, y0:y0 + 128, :])
                    vt.append(v)
                for c in range(C):
                    rt = io.tile([126, W], F32, tag="r", name=f"r{y0}_{c}")
                    nc.sync.dma_start(out=rt[:], in_=rgb_s[c, y0:y0 + 126, :])
                    dt = io.tile([126, W], F32, tag="d", name=f"d{y0}_{c}")
                    nc.sync.dma_start(out=dt[:], in_=d_s[c, y0:y0 + 126, :])

                    ps = psum.tile([128, W], F32, tag="ps", name=f"ps{y0}_{c}")
                    for xb in (0, 512):
                        for s in range(6):
                            i, kx = divmod(s, 3)
                            nc.tensor.matmul(
                                ps[:, xb:xb + 512],
                                band_sb[c][:, s * 128:(s + 1) * 128],
                                vt[i][:, xb + kx:xb + kx + 512],
                                start=(s == 0), stop=False)
                        nc.tensor.matmul(
                            ps[:126, xb:xb + 512],
                            diag_sb[c][:126, 0:126],
                            rt[:, xb:xb + 512], start=False, stop=False)
                        nc.tensor.matmul(
                            ps[:126, xb:xb + 512],
                            diag_sb[c][:126, 128:254],
                            dt[:, xb:xb + 512], start=False, stop=True)

                    diff = tmp.tile([126, W], F32, tag="diff", name=f"diff{y0}_{c}")
                    nc.vector.tensor_sub(out=diff[:], in0=rt[:], in1=dt[:])
                    prod = tmp.tile([126, W], F32, tag="prod", name=f"prod{y0}_{c}")
                    nc.vector.tensor_mul(out=prod[:], in0=diff[:], in1=ps[:126, :])
                    ot = io.tile([126, W], F32, tag="o", name=f"o{y0}_{c}")
                    nc.vector.tensor_add(out=ot[:], in0=prod[:], in1=dt[:])
                    nc.sync.dma_start(out=out_s[c, y0:y0 + 126, :], in_=ot[:])

            # ---------------- 4-row remainder, all channels stacked (m = 4c+r)
            vr = []
            for i in range(2):
                v = vpool.tile([6, W + 2], f, tag=f"vrem{i}", name=f"vrem{i}", bufs=1)
                nc.sync.dma_start(out=v[:], in_=var_s[i, REM_Y0:REM_Y0 + 6, :])
                vr.append(v)
            rr = io.tile([76, W], F32, tag="rrem", name="rrem", bufs=1)
            dr = io.tile([76, W], F32, tag="drem", name="drem", bufs=1)
            for c in range(C):
                nc.sync.dma_start(out=rr[4 * c:4 * c + 4, :],
                                  in_=rgb_s[c, REM_Y0:REM_Y0 + 4, :])
                nc.sync.dma_start(out=dr[4 * c:4 * c + 4, :],
                                  in_=d_s[c, REM_Y0:REM_Y0 + 4, :])
            ps = psum.tile([128, W], F32, tag="ps", name="ps_rem")
            for xb in (0, 512):
                for s in range(6):
                    i, kx = divmod(s, 3)
                    nc.tensor.matmul(
                        ps[:, xb:xb + 512],
                        remb_sb[:, s * 128:(s + 1) * 128],
                        vr[i][:, xb + kx:xb + kx + 512],
                        start=(s == 0), stop=False)
                nc.tensor.matmul(ps[:76, xb:xb + 512], remd_sb[:, 0:76],
                                 rr[:, xb:xb + 512], start=False, stop=False)
                nc.tensor.matmul(ps[:76, xb:xb + 512], remd_sb[:, 76:152],
                                 dr[:, xb:xb + 512], start=False, stop=True)
            diff = tmp.tile([76, W], F32, tag="diffrem", name="diff_rem", bufs=1)
            nc.vector.tensor_sub(out=diff[:], in0=rr[:], in1=dr[:])
            prod = tmp.tile([76, W], F32, tag="prodrem", name="prod_rem", bufs=1)
            nc.vector.tensor_mul(out=prod[:], in0=diff[:], in1=ps[:76, :])
            ot = io.tile([76, W], F32, tag="orem", name="o_rem", bufs=1)
            nc.vector.tensor_add(out=ot[:], in0=prod[:], in1=dr[:])
            for c in range(C):
                nc.sync.dma_start(out=out_s[c, REM_Y0:REM_Y0 + 4, :],
                                  in_=ot[4 * c:4 * c + 4, :])

    nc.compile()
    return nc


def _shard_inputs(rgb, d, rgb_var, d_var, W_prob, W_unc, W_total):
    bands, diags, remb, remd = _build_mats(
        np.asarray(W_prob, np.float32),
        np.asarray(W_unc, np.float32),
        np.asarray(W_total, np.float32))
    in_maps = []
    for core in range(NCORES):
        b, half = divmod(core, 2)
        h0 = half * R
        var = np.zeros((2, R + 2, W + 2), np.float32)
        lo, hi = max(h0 - 1, 0), min(h0 + R + 1, H)
        var[0, lo - h0 + 1:hi - h0 + 1, 1:W + 1] = rgb_var[b, 0, lo:hi, :]
        var[1, lo - h0 + 1:hi - h0 + 1, 1:W + 1] = d_var[b, 0, lo:hi, :]
        in_maps.append({
            "rgb_s": np.ascontiguousarray(rgb[b, :, h0:h0 + R, :], np.float32),
            "d_s": np.ascontiguousarray(d[b, :, h0:h0 + R, :], np.float32),
            "var_s": var,
            "bands": bands, "diags": diags, "remb": remb, "remd": remd,
        })
    return in_maps


def run(trace=False, **inputs):
    if "nc" not in _CACHE:
        _CACHE["nc"] = _build_program()
    nc = _CACHE["nc"]
    in_maps = _shard_inputs(**inputs)
    res = run_bass_kernel_spmd(nc, in_maps, list(range(NCORES)), trace=trace)
    out = np.empty((B, C, H, W), np.float32)
    for core in range(NCORES):
        b, half = divmod(core, 2)
        out[b, :, half * R:(half + 1) * R, :] = res.results[core]["out_s"]
    return out, res


def kernel(**inputs):
    out, _ = run(trace=False, **inputs)
    return out



# revision 20
# speedup vs baseline: 1.0848x; 1.0848x over previous
"""Trainium2 Bass kernel for ConditionalAttentionFusion-v2 (fp16, channel-packed).

Math (per batch b, channel c, pixel y,x):
    G    = a0[c]*rgb + a1[c]*d + conv3x3(CD, Wp[c])     CD = [rgb_var; d_var]
    out  = rgb*G + d*(1-G) = d + (rgb-d)*G
with a0 = Wt0*Wp0, a1 = Wt0*Wp1, Wp = Wt1*W_unc.

Strategy: pure data parallel over 8 cores (core = (batch, H-half), 256-row
slab).  All tensors travel as fp16 (graded gate is rel<2e-2; fp16 keeps us
~1e-3), halving HBM traffic to ~32MB/core (~105us at the 16x22.5B/ns DMA
fabric).

Layout: partition dim packs (row-in-block g, channel c): a 6-row block is
[114, 512] and ONE psum accumulation group computes G for all 19 channels:

    3 band matmuls  lhsT[40,114] x var[40,512]  (kx in free-shifts, ky+i in
                                                 the contraction dim)
    2 diag matmuls  diag(a0)/diag(a1) on rgb/d  [114,114] x [114,512]

All matmuls are fp16 1-pass (512 cycles): ~430 matmuls ~= 92us TensorE.
The var tile is [58, W+2]: map i at partitions 32i + (0..25), so one banded
lhsT per kx serves every block via partition-shifted rhs views.

ACT evacuates PSUM -> fp16 G (43us); DVE does 3 fp16 passes
q=rgb-d, p=q*G, out=p+d (~66us).  DMA instruction count is kept ~90 by
fusing 4 blocks per transfer with rearranged access patterns
("c (b g) x -> c g b x"); issue is split across SP (inputs/var/weights)
and ACT (d, outputs).
"""
import sys

if "/opt/trn_rl_repo" not in sys.path:
    sys.path.insert(0, "/opt/trn_rl_repo")

import numpy as np

import concourse.bacc as bacc
import concourse.mybir as mybir
import concourse.tile as tile
from concourse.bass_utils import run_bass_kernel_spmd

F32 = mybir.dt.float32
F16 = mybir.dt.float16
B, C, H, W = 4, 19, 512, 1024
R = 256              # slab rows per core
NCORES = 8
NS = 11              # supertiles: 10 x 24 rows + 1 x 16 rows
M6 = 6 * C           # 114: partitions of a 6-row block
M4 = 4 * C           # 76:  partitions of the 4-row tail block


# ----------------------------------------------------------------- host math
def _build_mats(W_prob, W_unc, W_total):
    a0 = (W_total[:, 0] * W_prob[:, 0]).astype(np.float32)
    a1 = (W_total[:, 0] * W_prob[:, 1]).astype(np.float32)
    Wp = W_total[:, 1][:, None, None, None] * W_unc          # [C,2,3,3]

    # band lhsT per (block-offset, kx): [58, c*gmax+g] with taps at
    # q = 32*i + off + g + ky.  rhs is always var_tile[0:58, ...] (matmul
    # requires base partition 0/32/64), so the block's row offset lives in
    # the lhsT.  psum partition is c-major: p = c*gmax + g, matching the
    # "c (b g) x -> c g b x" DMA rearrange.
    def band(gmax, offs):
        m = np.zeros((len(offs), 3, 58, gmax * C), np.float32)
        for o, off in enumerate(offs):
            for kx in range(3):
                for i in range(2):
                    for ky in range(3):
                        for g in range(gmax):
                            m[o, kx, 32 * i + off + g + ky, g::gmax] = Wp[:, i, ky, kx]
        return m

    def diag(gmax):
        n = gmax * C
        dg = np.zeros((n, 2 * n), np.float32)
        mm = np.arange(n)
        dg[mm, mm] = np.repeat(a0, gmax)
        dg[mm, n + mm] = np.repeat(a1, gmax)
        return dg

    b6 = band(6, (0, 6, 12, 18)).transpose(2, 0, 1, 3).reshape(58, 12 * M6)
    b4 = band(4, (12,)).transpose(2, 0, 1, 3).reshape(58, 3 * M4)
    return (b6.astype(np.float16), b4.astype(np.float16),
            diag(6).astype(np.float16), diag(4).astype(np.float16))


# ------------------------------------------------------------- bass program
_CACHE = {}


def _build_program():
    nc = bacc.Bacc("TRN2", debug=False, num_devices=NCORES)
    # rgb/d/out travel pre-transposed host-side as [C, g, block, W]
    # (row = 6*block + g, padded to 258 rows = 43 blocks) so one supertile
    # DMA is a 3-dim access pattern: (c, g, contiguous b*x).
    rgb_s = nc.dram_tensor("rgb_s", [C, 6, 43, W], F16, kind="ExternalInput").ap()
    d_s = nc.dram_tensor("d_s", [C, 6, 43, W], F16, kind="ExternalInput").ap()
    var_s = nc.dram_tensor("var_s", [2, R + 2, W + 2], F16, kind="ExternalInput").ap()
    wband = nc.dram_tensor("wband", [58, 12 * M6], F16, kind="ExternalInput").ap()
    wband4 = nc.dram_tensor("wband4", [58, 3 * M4], F16, kind="ExternalInput").ap()
    wdiag = nc.dram_tensor("wdiag", [M6, 2 * M6], F16, kind="ExternalInput").ap()
    wdiag4 = nc.dram_tensor("wdiag4", [M4, 2 * M4], F16, kind="ExternalInput").ap()
    out_s = nc.dram_tensor("out_s", [C, 6, 43, W], F16, kind="ExternalOutput").ap()

    with tile.TileContext(nc) as tc:
        with (
            tc.tile_pool(name="wpool", bufs=1) as wpool,
            tc.tile_pool(name="vvar", bufs=2) as vvar,
            tc.tile_pool(name="io", bufs=2) as io,
            tc.tile_pool(name="tmp", bufs=2) as tmp,
            tc.tile_pool(name="psum", bufs=3, space="PSUM") as psum,
        ):
            wb = wpool.tile([58, 12 * M6], F16, tag="wb", name="wb")
            nc.sync.dma_start(out=wb[:], in_=wband[:])
            wb4 = wpool.tile([58, 3 * M4], F16, tag="wb4", name="wb4")
            nc.sync.dma_start(out=wb4[:], in_=wband4[:])
            wd = wpool.tile([M6, 2 * M6], F16, tag="wd", name="wd")
            nc.sync.dma_start(out=wd[:], in_=wdiag[:])
            wd4 = wpool.tile([M4, 2 * M4], F16, tag="wd4", name="wd4")
            nc.sync.dma_start(out=wd4[:], in_=wdiag4[:])

            for s in range(NS):
                y0 = 24 * s
                nb = 4 if s < 10 else 2
                rows = 6 * nb
                # var rows incl. halo; s=10's tile also feeds the 4-row tail
                nf = 26 if s < 10 else 18

                # map0 over-reads to 32 rows so partitions 26..31 hold real
                # (zero-weighted) data — engines can't memset at base 26.
                nf0 = 32 if s < 10 else 18
                vt = vvar.tile([58, W + 2], F16, tag="var", name=f"var{s}")
                nc.sync.dma_start(out=vt[0:nf0, :], in_=var_s[0, y0:y0 + nf0, :])
                nc.sync.dma_start(out=vt[32:32 + nf, :], in_=var_s[1, y0:y0 + nf, :])

                rt = io.tile([M6, nb * W], F16, tag="r", name=f"r{s}")
                nc.sync.dma_start(out=rt[:], in_=rgb_s[:, :, 4 * s:4 * s + nb, :])
                dt = io.tile([M6, nb * W], F16, tag="d", name=f"d{s}")
                nc.scalar.dma_start(out=dt[:], in_=d_s[:, :, 4 * s:4 * s + nb, :])

                gt = tmp.tile([M6, nb * W], F16, tag="g", name=f"g{s}")
                for bi in range(nb):
                    ps = psum.tile([M6, W], F32, tag="ps", name=f"ps{s}_{bi}")
                    for xb in (0, 512):
                        for kx in range(3):
                            nc.tensor.matmul(
                                ps[:, xb:xb + 512],
                                wb[:, (bi * 3 + kx) * M6:(bi * 3 + kx + 1) * M6],
                                vt[0:58, xb + kx:xb + kx + 512],
                                start=(kx == 0), stop=False)
                        nc.tensor.matmul(
                            ps[:, xb:xb + 512], wd[:, 0:M6],
                            rt[:, bi * W + xb:bi * W + xb + 512],
                            start=False, stop=False)
                        nc.tensor.matmul(
                            ps[:, xb:xb + 512], wd[:, M6:2 * M6],
                            dt[:, bi * W + xb:bi * W + xb + 512],
                            start=False, stop=True)
                    nc.scalar.copy(out=gt[:, bi * W:(bi + 1) * W], in_=ps[:])

                qt = tmp.tile([M6, nb * W], F16, tag="q", name=f"q{s}")
                nc.vector.tensor_sub(out=qt[:], in0=rt[:], in1=dt[:])
                pt = tmp.tile([M6, nb * W], F16, tag="p", name=f"p{s}")
                nc.vector.tensor_mul(out=pt[:], in0=qt[:], in1=gt[:])
                ot = io.tile([M6, nb * W], F16, tag="o", name=f"o{s}")
                nc.vector.tensor_add(out=ot[:], in0=pt[:], in1=dt[:])
                nc.scalar.dma_start(out=out_s[:, :, 4 * s:4 * s + nb, :], in_=ot[:])

            # ---- 4-row tail (rows 252..255), shares s=10's var tile ----
            r4 = io.tile([M4, W], F16, tag="r4", name="r4", bufs=1)
            nc.sync.dma_start(out=r4[:], in_=rgb_s[:, 0:4, 42, :])
            d4 = io.tile([M4, W], F16, tag="d4", name="d4", bufs=1)
            nc.scalar.dma_start(out=d4[:], in_=d_s[:, 0:4, 42, :])
            g4 = tmp.tile([M4, W], F16, tag="g4", name="g4", bufs=1)
            ps4 = psum.tile([M4, W], F32, tag="ps4", name="ps4", bufs=1)
            for xb in (0, 512):
                for kx in range(3):
                    nc.tensor.matmul(
                        ps4[:, xb:xb + 512],
                        wb4[:, kx * M4:(kx + 1) * M4],
                        vt[0:58, xb + kx:xb + kx + 512],
                        start=(kx == 0), stop=False)
                nc.tensor.matmul(ps4[:, xb:xb + 512], wd4[:, 0:M4],
                                 r4[:, xb:xb + 512], start=False, stop=False)
                nc.tensor.matmul(ps4[:, xb:xb + 512], wd4[:, M4:2 * M4],
                                 d4[:, xb:xb + 512], start=False, stop=True)
            nc.scalar.copy(out=g4[:], in_=ps4[:])
            q4 = tmp.tile([M4, W], F16, tag="q4", name="q4", bufs=1)
            nc.vector.tensor_sub(out=q4[:], in0=r4[:], in1=d4[:])
            p4 = tmp.tile([M4, W], F16, tag="p4", name="p4", bufs=1)
            nc.vector.tensor_mul(out=p4[:], in0=q4[:], in1=g4[:])
            o4 = io.tile([M4, W], F16, tag="o4", name="o4", bufs=1)
            nc.vector.tensor_add(out=o4[:], in0=p4[:], in1=d4[:])
            nc.scalar.dma_start(out=out_s[:, 0:4, 42, :], in_=o4[:])

    nc.compile()
    return nc


def _shard_inputs(rgb, d, rgb_var, d_var, W_prob, W_unc, W_total):
    b6, b4, dg6, dg4 = _build_mats(
        np.asarray(W_prob, np.float32),
        np.asarray(W_unc, np.float32),
        np.asarray(W_total, np.float32))
    rgb16 = np.asarray(rgb, np.float32).astype(np.float16)
    d16 = np.asarray(d, np.float32).astype(np.float16)
    rv = np.asarray(rgb_var, np.float32)
    dv = np.asarray(d_var, np.float32)

    def relayout(slab):                       # [C, 256, W] -> [C, 6, 43, W]
        p = np.zeros((C, 258, W), np.float16)
        p[:, :R, :] = slab
        return np.ascontiguousarray(
            p.reshape(C, 43, 6, W).transpose(0, 2, 1, 3))

    in_maps = []
    for core in range(NCORES):
        b, half = divmod(core, 2)
        h0 = half * R
        var = np.zeros((2, R + 2, W + 2), np.float16)
        lo, hi = max(h0 - 1, 0), min(h0 + R + 1, H)
        var[0, lo - h0 + 1:hi - h0 + 1, 1:W + 1] = rv[b, 0, lo:hi, :]
        var[1, lo - h0 + 1:hi - h0 + 1, 1:W + 1] = dv[b, 0, lo:hi, :]
        in_maps.append({
            "rgb_s": relayout(rgb16[b, :, h0:h0 + R, :]),
            "d_s": relayout(d16[b, :, h0:h0 + R, :]),
            "var_s": var,
            "wband": b6, "wband4": b4, "wdiag": dg6, "wdiag4": dg4,
        })
    return in_maps


def run(trace=False, **inputs):
    if "nc" not in _CACHE:
        _CACHE["nc"] = _build_program()
    nc = _CACHE["nc"]
    in_maps = _shard_inputs(**inputs)
    res = run_bass_kernel_spmd(nc, in_maps, list(range(NCORES)), trace=trace)
    out = np.empty((B, C, H, W), np.float32)
    for core in range(NCORES):
        b, half = divmod(core, 2)
        o = res.results[core]["out_s"]        # [C, 6, 43, W]
        o = o.transpose(0, 2, 1, 3).reshape(C, 258, W)[:, :R, :]
        out[b, :, half * R:(half + 1) * R, :] = o.astype(np.float32)
    return out, res


def kernel(**inputs):
    out, _ = run(trace=False, **inputs)
    return out
